# revision 26
# baseline (speedup 1.0000x reference)
"""Trainium2 Bass kernel for nn_MultiHeadedAttention — transposed dataflow, v2.

Scores are computed TRANSPOSED: S^T[k, q] = (c_k kd_hat).(a_q qd_hat), with all
norm/scale factors folded into the projected direction vectors (a = S*qn/|qd|,
c = S*kn/|kd|, S = 10/32^0.25). A per-query softmax shift m_q rides the score
matmul as an augmented contraction row (K=33): k-side aux row = 1, q-side aux
row carries the shift. m_q = LAM*|S*qn_q|*RMS_k(S*kn) is a statistical upper
bound on the row max (validated: allmax_q - 85 <= m_q <= unmasked_max_q + 78
for this model's input distribution), so softmax shift-invariance makes it
range-safe without a max pass.

v2 changes vs v1 (cost-model driven):
 1. All score-path matmuls run as float32r (11-bit-mantissa fp32, full
    1 col/cycle PE rate at N>=512) instead of float32 (4x slower). The PE
    busy time for scores drops 874us -> 218us per core.
 2. The PE emits t = A16*(s - m_q) + B16 directly (A16 = 128/ln2 folded into
    the q-side scaling, B16 into the aux row). One PSUM tile then feeds two
    exp routes:
      - ACT route: exp via activation(Exp, scale=1/A16, bias=-B16/A16) —
        exact, the affine is free.
      - DVE route: a SINGLE fused TensorTensor op computes
        uint16(sat(round(t * mask))), whose bits ARE the bf16 Schraudolph
        approximation of mask*e^(s-m_q): masked or underflowed entries
        saturate to 0, everything else becomes 2^((u-B16)/128). Measured
        end-to-end error of this approximation on the real data: <=1.1e-3
        even at 100% usage (softmax ratio cancels the sawtooth).
    Routing is a static kc pattern tuned to balance ACT/DVE/Pool busy time.
 3. The mask multiply for ACT-route tiles runs on DVE (bf16 2x) and partially
    on GPSIMD (Pool) to keep DVE under the PE roofline.
 4. Projection-phase matmuls are float32r as well; the b0 bias rides the
    activation bias port ([128,1] AP) instead of a rank-1 matmul.

Cost-model (CoreSim) per-core budget: PE ~494us (scores 218 + PV 218 + proj
~56), ACT ~486us, DVE ~472us, Pool ~145us, DMA ~65us -> ~510us total vs
~1414us for v1.

Sharding: core c -> batch b=c//2, query-half c%2 (mask read exactly once).
"""

import numpy as np

import concourse.bass as bass
import concourse.mybir as mybir
from concourse import bacc
from concourse.tile import TileContext
from concourse import bass_utils
from concourse.masks import make_identity

F32 = mybir.dt.float32
F32R = mybir.dt.float32r
BF16 = mybir.dt.bfloat16
U16 = mybir.dt.uint16
I32 = mybir.dt.int32
AF = mybir.ActivationFunctionType
OP = mybir.AluOpType

B, SQ, SK, D, H, DK = 4, 4096, 4096, 256, 8, 32
NCORES = 8
R = SQ // 2          # q rows per core
QH = R // 1024       # 2 q-half blocks of 1024
KT = SK // 128       # 32 k-tiles of 128
SCALE = 10.0 / (32.0 ** 0.25)
LAM = 1.51           # shift coefficient, window [1.36, 1.66] w/ margins (85,78)
A16 = 128.0 / np.log(2.0)          # Schraudolph exponent scale (bf16 frame)
B16 = 16255.5                      # 127*128 + sigma (sigma=-0.5, RN convert)
EXP_SCALE = float(1.0 / A16)
EXP_BIAS = float(-B16 / A16)

# kc-routing patterns (per 16 kc): fused DVE Schraudolph tiles, and which of
# the ACT-route tiles get their mask multiply on GPSIMD instead of DVE.
DVE_PAT = frozenset((1, 4, 7, 10, 12, 15))   # 12 of 32 -> DVE-fused
POOL_PAT = frozenset((0, 3, 6, 9, 11, 14))   # 12 of 32 -> Pool mask-mul

_CACHE = {}


def _build(repeat=1):
    if repeat in _CACHE:
        return _CACHE[repeat]
    nc = bacc.Bacc("TRN2", target_bir_lowering=False, debug=False,
                   num_devices=NCORES)

    q_d = nc.dram_tensor("q", [R, D], F32R, kind="ExternalInput")
    k_d = nc.dram_tensor("k", [SK, D], F32R, kind="ExternalInput")
    v_d = nc.dram_tensor("v", [1, SK], F32, kind="ExternalInput")
    mt_d = nc.dram_tensor("mt", [SK, R], I32, kind="ExternalInput")
    # w0p: outc-permuted+padded w0.T -> [inc, 4 groups x 128]
    w0p_d = nc.dram_tensor("w0p", [D, 4 * 128], F32R, kind="ExternalInput")
    w1t8_d = nc.dram_tensor("w1t8", [D, H], F32R, kind="ExternalInput")
    b0pt_d = nc.dram_tensor("b0pt", [128, 4], F32, kind="ExternalInput")
    b18_d = nc.dram_tensor("b18", [1, H], F32R, kind="ExternalInput")
    inds_d = nc.dram_tensor("inds", [128, 4 * H], F32R, kind="ExternalInput")
    indst_d = nc.dram_tensor("indst", [H, 4 * 128], F32R, kind="ExternalInput")
    out_d = nc.dram_tensor("o", [QH, 1024], F32, kind="ExternalOutput")

    with TileContext(nc) as tc:
        with tc.tile_pool(name="persist", bufs=1) as pp:
            identf = pp.tile([128, 128], F32, tag="identf")
            make_identity(nc, identf[:])
            ident = pp.tile([128, 128], F32R, tag="ident")
            nc.vector.tensor_scalar(out=ident[:], in0=identf[:], scalar1=1.0,
                                    scalar2=None, op0=OP.mult)
            w0p = pp.tile([128, 2, 4, 128], F32R, tag="w0p")
            nc.sync.dma_start(w0p[:], w0p_d.rearrange("(a p) (g o) -> p a g o",
                                                      p=128, g=4))
            w1t8 = pp.tile([128, 2, H], F32R, tag="w1t8")
            nc.sync.dma_start(w1t8[:], w1t8_d.rearrange("(a p) o -> p a o", p=128))
            b0pt = pp.tile([128, 4], F32, tag="b0pt")
            nc.sync.dma_start(b0pt[:], b0pt_d[:, :])
            b18 = pp.tile([1, H], F32R, tag="b18")
            nc.sync.dma_start(b18[:], b18_d[:])
            inds = pp.tile([128, 4, H], F32R, tag="inds")
            nc.sync.dma_start(inds[:], inds_d.rearrange("p (g o) -> p g o", g=4))
            indst = pp.tile([H, 4, 128], F32R, tag="indst")
            nc.sync.dma_start(indst[:], indst_d.rearrange("p (g o) -> p g o", g=4))
            expb = pp.tile([128, 1], F32, tag="expb")
            nc.gpsimd.memset(expb[:], EXP_BIAS)
            ones_f = pp.tile([1, 512], F32, tag="ones_f")
            nc.gpsimd.memset(ones_f[:], 1.0)
            ones_row = pp.tile([1, 512], F32R, tag="ones_row")
            nc.vector.tensor_scalar(out=ones_row[:], in0=ones_f[:], scalar1=1.0,
                                    scalar2=None, op0=OP.mult)

            # [v | 1] stationary operands for the PV matmul, per k-tile
            uvt = pp.tile([128, KT, 2], BF16, tag="uvt")
            nc.gpsimd.dma_start(uvt[:, :, 0],
                                v_d.rearrange("a (c p) -> p (a c)", p=128))
            nc.gpsimd.memset(uvt[:, :, 1:2], 1.0)

            # projected tensors, augmented layout:
            # group gp: head dims at rows 64*u..+32, aux row at 32/96
            qdT = pp.tile([128, 4, R], F32R, tag="qdT")
            kdT = pp.tile([128, 4, SK], F32R, tag="kdT")
            # early mask quarters: two slots fit alongside the projection
            # pools; their DMAs run during the projection phase.
            mallE_ctx = tc.tile_pool(name="mallE", bufs=2)
            mallE = mallE_ctx.__enter__()
            mtq = {}
            for qh, kq in ((0, 0), (0, 1), (1, 0), (1, 1)):
                mt = mallE.tile([128, 8, 1024], BF16, tag="mqE")
                nc.gpsimd.dma_start(
                    mt[:],
                    mt_d[kq * 1024:(kq + 1) * 1024,
                         qh * 1024:(qh + 1) * 1024].rearrange(
                        "(c p) q -> p c q", p=128))
                mtq[(qh, kq)] = mt
            shp_ctx = tc.tile_pool(name="shp", bufs=1)
            shp = shp_ctx.__enter__()
            mq = shp.tile([8, R], F32, tag="mq")         # SCALE*|qn|
            sskp = shp.tile([8, 8], F32, tag="sskp")     # per-chunk sum kn'^2

            # k-side aux rows (constant 1.0, f32r): build a [1,2048] f32r ones
            # row, then two cheap broadcast DMAs per (gp, u) row — all early,
            # before the projection writes land (disjoint partition rows).
            ones2kr = shp.tile([1, 2048], F32R, tag="ones2kr")
            for i in range(4):
                nc.vector.tensor_scalar(out=ones2kr[0:1, i * 512:(i + 1) * 512],
                                        in0=ones_f[:], scalar1=1.0,
                                        scalar2=None, op0=OP.mult)
            for gp in range(4):
                for rr in (32, 96):
                    for hx in range(2):
                        nc.gpsimd.dma_start(
                            kdT[rr:rr + 1, gp, hx * 2048:(hx + 1) * 2048],
                            ones2kr[0:1, :])


            with (
                tc.tile_pool(name="nat", bufs=3) as natp,
                tc.tile_pool(name="xT", bufs=2) as xTp,
                tc.tile_pool(name="psT", bufs=2, space="PSUM") as psT,
                tc.tile_pool(name="psP", bufs=2, space="PSUM") as psP,
                tc.tile_pool(name="psS", bufs=1, space="PSUM") as psS,
                tc.tile_pool(name="psE", bufs=2, space="PSUM") as psE,
                tc.tile_pool(name="sq", bufs=1) as sqp,
                tc.tile_pool(name="sm", bufs=1) as smp,
            ):
                chunks = ([(q_d, qdT, True, ch) for ch in range(R // 512)]
                          + [(k_d, kdT, False, ch) for ch in range(SK // 512)])
                for src_d, xdT, is_q, ch in chunks:
                    # q side folds A16 into the direction scaling (srt scale)
                    s0 = 1.0 / (SCALE * SCALE * (A16 * A16 if is_q else 1.0))
                    if True:
                        cs = slice(ch * 512, (ch + 1) * 512)
                        xT = xTp.tile([128, 2, 512], F32R, tag="xT")
                        for rt in range(4):
                            nat = natp.tile([128, D], F32R, tag="nat")
                            r0 = ch * 512 + rt * 128
                            nc.sync.dma_start(nat[:], src_d[r0:r0 + 128, :])
                            for kc in range(2):
                                pt = psT.tile([128, 128], F32R, tag="pt")
                                nc.tensor.transpose(
                                    pt[:], nat[:, kc * 128:(kc + 1) * 128], ident[:])
                                nc.scalar.copy(
                                    xT[:, kc, rt * 128:(rt + 1) * 128], pt[:])
                        # norms projection qn[8, 512] (+bias via b18 matmul)
                        pn = psS.tile([8, 512], F32, tag="pn")
                        for kc in range(2):
                            nc.tensor.matmul(pn[:], w1t8[:, kc, :], xT[:, kc, :],
                                             start=(kc == 0), stop=False)
                        nc.tensor.matmul(pn[:], b18[0:1, :], ones_row[0:1, :],
                                         start=False, stop=True)
                        if is_q:
                            # mq = SCALE*|qn| (abs via Abs activation)
                            nc.scalar.activation(
                                mq[:, cs], pn[:], AF.Abs, scale=SCALE)
                        elif ch < 4:
                            # RMS(kn) from the first 2048 keys only: the
                            # LAM window has >=10% slack vs the ~1.5% spread
                            # of a 2048-sample RMS, and this unblocks the
                            # q-side aux rows before the k projection ends.
                            sqn = smp.tile([8, 512], F32, tag="sqn")
                            nc.scalar.square(sqn[:], pn[:])
                            nc.vector.tensor_reduce(
                                sskp[:, ch:ch + 1], sqn[:],
                                axis=mybir.AxisListType.X,
                                op=mybir.AluOpType.add)
                        # per-group direction projections + scaling
                        sq_ = [None] * 4
                        rw_ = [None] * 4
                        for gp in range(4):
                            pr = psP.tile([128, 512], F32, tag="pr")
                            for kc in range(2):
                                nc.tensor.matmul(
                                    pr[:], w0p[:, kc, gp, :], xT[:, kc, :],
                                    start=(kc == 0), stop=(kc == 1))
                            # bias rides the ACT bias port; sq in f32r for the
                            # fp32r pss matmul
                            sq_[gp] = sqp.tile([128, 512], F32R, tag=f"sq{gp}",
                                               name=f"sq{gp}")
                            nc.scalar.activation(sq_[gp][:], pr[:], AF.Square,
                                                 bias=b0pt[:, gp:gp + 1],
                                                 scale=1.0)
                            rw_[gp] = sqp.tile([128, 512], F32, tag=f"rw{gp}",
                                               name=f"rw{gp}")
                            nc.scalar.activation(rw_[gp][:], pr[:], AF.Identity,
                                                 bias=b0pt[:, gp:gp + 1],
                                                 scale=1.0)
                        pss = psS.tile([8, 512], F32, tag="pss")
                        for gp in range(4):
                            nc.tensor.matmul(pss[:], inds[:, gp, :], sq_[gp][:],
                                             start=(gp == 0), stop=(gp == 3))
                        srt = smp.tile([8, 512], F32, tag="srt")
                        nc.scalar.activation(srt[:], pss[:], AF.Sqrt, scale=s0)
                        rn = smp.tile([8, 512], F32, tag="rn")
                        nc.vector.reciprocal_approx_fast(rn[:], srt[:])
                        av = smp.tile([8, 512], F32R, tag="av")
                        nc.vector.tensor_mul(av[:], pn[:], rn[:])
                        for gp in range(4):
                            pe = psE.tile([128, 512], F32, tag="pe")
                            nc.tensor.matmul(pe[:], indst[:, gp, :], av[:])
                            for u in range(2):
                                nc.vector.tensor_mul(
                                    xdT[64 * u:64 * u + 32, gp, cs],
                                    rw_[gp][64 * u:64 * u + 32, :],
                                    pe[64 * u:64 * u + 32, :])

            # q-side aux = -A16*m_q + B16, m_q = mq * tsh,
            # tsh = LAM*SCALE*RMS(SCALE*kn) = sqrt(ssk * LAM^2*SCALE^2/SK)
            ssk = shp.tile([8, 1], F32, tag="ssk")
            nc.vector.tensor_reduce(ssk[:], sskp[:, 0:4],
                                    axis=mybir.AxisListType.X,
                                    op=mybir.AluOpType.add)
            tshA = shp.tile([8, 1], F32, tag="tshA")
            nc.scalar.activation(
                tshA[:], ssk[:], AF.Sqrt,
                scale=float(A16 * A16 * LAM * LAM * SCALE * SCALE / (SK // 2)))
            tshn = shp.tile([8, 1], F32, tag="tshn")
            nc.vector.tensor_scalar(out=tshn[:], in0=tshA[:], scalar1=-1.0,
                                    scalar2=None, op0=OP.mult)
            aux = shp.tile([8, R], F32R, tag="aux")
            nc.vector.tensor_scalar(out=aux[:], in0=mq[:], scalar1=tshn[:],
                                    scalar2=float(B16), op0=OP.mult, op1=OP.add)
            # distribute aux rows into qdT (u-major norms layout, host
            # permutation). Single-partition DMAs are per-partition-rate
            # bound, so split per (u, gp) across two HWDGE queues.
            for u in range(2):
                for gp in range(4):
                    eng = nc.sync if gp % 2 == 0 else nc.scalar
                    eng.dma_start(qdT[32 + 64 * u:33 + 64 * u, gp, :],
                                  aux[4 * u + gp:4 * u + gp + 1, :])

            shp_ctx.__exit__(None, None, None)

            # ---- main attention loop (transposed scores, t-frame) ----
            with (
                tc.tile_pool(name="mall", bufs=2) as mallp,
                tc.tile_pool(name="psSc", bufs=3, space="PSUM") as psc,
                tc.tile_pool(name="psNd", bufs=1, space="PSUM") as psnd,
                tc.tile_pool(name="ebuf", bufs=2) as ebufp,
                tc.tile_pool(name="etl", bufs=4) as etlp,
                tc.tile_pool(name="sm2", bufs=1) as sm2p,
                tc.tile_pool(name="xacc", bufs=1) as xaccp,
            ):
                for _rep in range(repeat):
                    # late mask quarters (kq 2,3): qh0's issued here, qh1's at
                    # its own section to keep the Pool wait-queue under depth 4
                    def issue_mask(qh, kq):
                        mt = mallp.tile([128, 8, 1024], BF16, tag="mqL")
                        nc.gpsimd.dma_start(
                            mt[:],
                            mt_d[kq * 1024:(kq + 1) * 1024,
                                 qh * 1024:(qh + 1) * 1024].rearrange(
                                "(c p) q -> p c q", p=128))
                        mtq[(qh, kq)] = mt

                    issue_mask(0, 2)
                    issue_mask(0, 3)
                    for qh in range(QH):
                        if qh == 1:
                            issue_mask(1, 2)
                            issue_mask(1, 3)
                        q0 = qh * 1024
                        xas = [xaccp.tile([1, 1024], F32, tag=f"xa{i % 2}",
                                          name=f"xa{i % 2}") for i in range(H + 1)]
                        nc.gpsimd.memset(xas[0][:], 0.0)
                        for h in range(H):
                            gp, u = divmod(h, 2)
                            r0 = 64 * u
                            nd = psnd.tile([2, 1024], F32, tag="nd")
                            LAG = 3   # PV matmuls trail scores by LAG k-tiles
                            ets = {}

                            def issue_pv(kc):
                                for j in range(2):
                                    nc.tensor.matmul(
                                        nd[:, j * 512:(j + 1) * 512],
                                        uvt[:, kc, :],
                                        ets[kc][:, j * 512:(j + 1) * 512],
                                        start=(kc == 0), stop=(kc == KT - 1))
                                del ets[kc]

                            for kc in range(KT):
                                ps = psc.tile([128, 1024], F32, tag="ps")
                                lhsT = kdT[r0:r0 + 33, gp,
                                           kc * 128:(kc + 1) * 128]
                                for j in range(2):
                                    nc.tensor.matmul(
                                        ps[:, j * 512:(j + 1) * 512], lhsT,
                                        qdT[r0:r0 + 33, gp,
                                            q0 + j * 512:q0 + (j + 1) * 512],
                                        tile_position=(r0, 0))
                                mask = mtq[(qh, kc // 8)][:, kc % 8, :]
                                et = etlp.tile([128, 1024], BF16, tag="et")
                                kk = kc % 16
                                if kk in DVE_PAT:
                                    # fused mask*Schraudolph-exp: u16 bits are
                                    # the bf16 approximation of mask*e^(s-m_q)
                                    nc.vector.tensor_tensor(
                                        out=et[:].bitcast(U16), in0=ps[:],
                                        in1=mask, op=OP.mult)
                                else:
                                    e = ebufp.tile([128, 1024], BF16, tag="e")
                                    nc.scalar.activation(
                                        e[:], ps[:], AF.Exp,
                                        scale=EXP_SCALE, bias=expb[:, 0:1])
                                    if kk in POOL_PAT:
                                        nc.gpsimd.tensor_mul(et[:], e[:], mask)
                                    else:
                                        nc.vector.tensor_mul(et[:], e[:], mask)
                                ets[kc] = et
                                if kc >= LAG:
                                    issue_pv(kc - LAG)
                            for kc in range(KT - LAG, KT):
                                issue_pv(kc)
                            ndc = sm2p.tile([2, 1024], F32, tag="ndc")
                            nc.scalar.copy(ndc[:], nd[:])
                            dent = sm2p.tile([1, 1024], F32, tag="dent")
                            nc.sync.dma_start(dent[:], ndc[1:2, :])
                            rden = sm2p.tile([1, 1024], F32, tag="rden")
                            nc.vector.reciprocal_approx_fast(rden[:], dent[:])
                            xh = sm2p.tile([1, 1024], F32, tag="xh")
                            nc.vector.tensor_mul(xh[:], ndc[0:1, :], rden[:])
                            nc.vector.tensor_add(xas[h + 1][:], xas[h][:], xh[:])
                        oof = sm2p.tile([1, 1024], F32, tag="xh", name="oof")
                        nc.scalar.mul(oof[:], xas[H][:], 1.0 / H)
                        nc.sync.dma_start(out_d[qh:qh + 1, :], oof[:])

            mallE_ctx.__exit__(None, None, None)

    nc.finalize()
    _CACHE[repeat] = nc
    return nc


def _prep_host(query, key, value, mask, w0, b0, w1, b1):
    # outc permutation: group gp = h//2 holds head 2gp at rows 0-31 and head
    # 2gp+1 at rows 64-95; rows 32-63/96-127 are zero padding (row 32/96 later
    # becomes the augmented shift row on device).
    w0p = np.zeros((D, 4 * 128), np.float32)
    b0pt = np.zeros((128, 4), np.float32)
    inds = np.zeros((128, 4 * H), np.float32)
    indst = np.zeros((H, 4 * 128), np.float32)
    w0t = w0.T.astype(np.float32)            # [inc, outc]
    # norms-side rows use a u-major head order j = u*4 + gp (head 2gp+u) so
    # the on-device aux rows for each u are partition-contiguous.
    w1t8 = np.zeros((D, H), np.float32)
    b18 = np.zeros((1, H), np.float32)
    for h in range(H):
        gp, u = divmod(h, 2)
        j = u * 4 + gp
        dst = gp * 128 + 64 * u
        w0p[:, dst:dst + 32] = w0t[:, 32 * h:32 * h + 32]
        b0pt[64 * u:64 * u + 32, gp] = b0[32 * h:32 * h + 32]
        inds[64 * u:64 * u + 32, gp * H + j] = 1.0
        indst[j, gp * 128 + 64 * u:gp * 128 + 64 * u + 32] = 1.0
        w1t8[:, j] = w1[h]
        b18[0, j] = b1[h]
    in_maps = []
    for c in range(NCORES):
        b, half = divmod(c, 2)
        r0 = half * R
        in_maps.append({
            "q": np.ascontiguousarray(query[b, r0:r0 + R]),
            "k": np.ascontiguousarray(key[b]),
            "v": np.ascontiguousarray(value[b].reshape(1, SK)),
            "mt": np.ascontiguousarray(mask[b, r0:r0 + R].T),
            "w0p": w0p, "w1t8": w1t8, "b0pt": b0pt, "b18": b18,
            "inds": inds, "indst": indst,
        })
    return in_maps


def kernel(query, key, value, mask, w0, b0, w1, b1, _repeat=1):
    query = np.asarray(query, np.float32)
    key = np.asarray(key, np.float32)
    value = np.asarray(value, np.float32)
    mask = np.asarray(mask, np.int32)
    nc = _build(_repeat)
    in_maps = _prep_host(query, key, value, mask, w0, b0, w1, b1)
    res = bass_utils.run_bass_kernel_spmd(nc, in_maps, core_ids=list(range(NCORES)))
    out = np.empty((B, SQ, 1), np.float32)
    for c in range(NCORES):
        b, half = divmod(c, 2)
        out[b, half * R:(half + 1) * R, 0] = res.results[c]["o"].reshape(R)
    return out
